# revision 28
# baseline (speedup 1.0000x reference)
"""Causal self-attention on 8 Trainium2 NeuronCores.

Tensor-parallel by heads: each core owns 2 of the 16 heads end-to-end
(QKV projection -> causal attention -> row-sharded output projection),
and the 8 partial projection outputs are summed on the host.

Layout notes (per core):
  - x is pre-transposed on host to xT [C, B*T] so the contraction dim C
    lands on SBUF partitions with no on-device transpose of x.
  - QKV weights are sharded column-wise and reordered to
    [qA qB | kA kB | vA vB] (A/B = the core's two heads), so the
    qkvT = W^T @ xT matmuls directly produce QT/KT/VT with head A on
    partitions 0-63 and head B on partitions 64-127.
  - V is PE-transposed back to [token, feat]; head A gets an extra ones
    column (VA = [vA | 1], M=65) and head B is stored as
    [0..0 | 1 | vB] (M=128) so that the attnV matmuls produce, in one
    accumulation chain, both the unnormalized y^T rows *and* the softmax
    denominators at the partition rows where they are needed.
  - scoresT[kpos, qpos] = K^T-block^T-free form; softmax needs no max
    subtraction (inputs are well-scaled); causal mask applied to exp
    tiles via gpsimd affine_select (index predicate, no mask tensor).
  - proj: out_partial = y_shard @ Wp_shard; V-bias and b_proj are folded
    into a host-precomputed per-core row bias (attn rows sum to 1).
"""

import os
import numpy as np
from contextlib import ExitStack

import concourse.bass as bass
import concourse.mybir as mybir
import concourse.tile as tile
from concourse import bacc

B, T, C, H, D = 2, 2048, 1024, 16, 64
NCORES = 8
HPC = H // NCORES          # heads per core = 2
BT = B * T                 # 4096 tokens
P = 128
KO = C // P                # 8 contraction chunks of 128
MT = 512                   # qkv m-tile (tokens)
NMT_B = T // MT            # 4 m-tiles per batch
QTW = 512                  # q tile width
NQT = T // QTW             # 4
NKB = T // P               # 16 k-blocks per batch
SCALE = 1.0 / np.sqrt(D)   # 0.125
F32 = mybir.dt.float32
F32R = mybir.dt.float32r

# dtype for every tensor that feeds the PE. bf16 streams at 1 cycle/row
# (vs 4 for fp32) and supports the normal ldweights path + FWL; fp32r is
# unusable (codegen limits self-loading matmuls to one sync wait).
BF16 = mybir.dt.bfloat16
_MM = os.environ.get("KERNEL_MMDT", "bf16")
MMDT = {"bf16": BF16, "f32": F32}[_MM]

LAST_RESULT = None  # BassKernelResults of the most recent run (for profiling)


def build_nc():
    nc = bacc.Bacc(target_bir_lowering=False)

    xT_d = nc.dram_tensor("xT", [C, BT], MMDT, kind="ExternalInput")
    w_d = nc.dram_tensor("w", [C, 3 * HPC * D], MMDT, kind="ExternalInput")
    bqk_d = nc.dram_tensor("bqk", [P, 2], F32, kind="ExternalInput")
    wp_d = nc.dram_tensor("wp", [P, C], MMDT, kind="ExternalInput")
    id_d = nc.dram_tensor("ident", [P, P], F32, kind="ExternalInput")
    out_d = nc.dram_tensor("out", [BT, C], BF16, kind="ExternalOutput")

    xT3 = xT_d.ap().rearrange("(ko p) m -> p ko m", p=P)   # [128, 8, 4096]
    w3 = w_d.ap().rearrange("(ko p) n -> p ko n", p=P)     # [128, 8, 384]
    out2 = out_d.ap()                                       # [4096, 1024]

    Exp = mybir.ActivationFunctionType.Exp
    mult = mybir.AluOpType.mult
    add = mybir.AluOpType.add

    with tile.TileContext(nc) as tc, ExitStack() as ctx:
        const = ctx.enter_context(tc.tile_pool(name="const", bufs=1))
        big = ctx.enter_context(tc.tile_pool(name="big", bufs=1))
        xpool = ctx.enter_context(tc.tile_pool(name="xpool", bufs=2))
        epool = ctx.enter_context(tc.tile_pool(name="epool", bufs=3))
        spool = ctx.enter_context(tc.tile_pool(name="spool", bufs=2))
        opool = ctx.enter_context(tc.tile_pool(name="opool", bufs=6))
        ypool = ctx.enter_context(tc.tile_pool(name="ypool", bufs=8))
        dpool = ctx.enter_context(tc.tile_pool(name="dpool", bufs=2, space="DRAM"))
        psum = ctx.enter_context(tc.tile_pool(name="psum", bufs=1, space="PSUM"))

        # ---------------- constants ----------------
        # w rides the sync queue (followed immediately by the xm loads),
        # split per contraction chunk so the first QKV matmul only waits for
        # its own k=0 slices; the smaller consts go via the scalar queue so
        # they don't delay the first QKV matmul's inputs.
        w_sb = const.tile([P, KO, 3 * HPC * D], MMDT)
        wp_sb = const.tile([P, C], MMDT)
        nc.scalar.dma_start(out=wp_sb[:, :], in_=wp_d.ap()[:, :])
        id_sb = const.tile([P, P], F32)
        nc.scalar.dma_start(out=id_sb[:], in_=id_d.ap())
        bqk_sb0 = const.tile([P, 2], F32)
        nc.scalar.dma_start(out=bqk_sb0[:], in_=bqk_d.ap())
        # Pre-consume DMA semaphores on the engines that will read these
        # tiles: same-engine deps need no sync wait, and several encodings
        # (TensorScalarPtr, fp32 self-loading matmul) only have ONE wait
        # slot, so a (PE, DMA) wait pair fails walrus codegen.
        bqk_sb = const.tile([P, 2], F32)
        nc.vector.tensor_copy(out=bqk_sb[:], in_=bqk_sb0[:])
        idb_sb = const.tile([P, P], MMDT)
        nc.vector.tensor_copy(out=idb_sb[:], in_=id_sb[:])
        pid = psum.tile([P, P], F32, tag="py", bufs=4)
        nc.tensor.transpose(pid[:], id_sb[:], id_sb[:])
        pwp = psum.tile([P, QTW], F32, tag="py", bufs=4)
        nc.tensor.matmul(pwp[:, 0:P], wp_sb[:, 0:P], wp_sb[:, 0:P],
                         start=True, stop=True)
        nc.tensor.matmul(pwp[:, 0:P], wp_sb[:, QTW:QTW + P],
                         wp_sb[:, QTW:QTW + P], start=True, stop=True)

        QT_sb = big.tile([P, B, T], MMDT)   # rows: [qA feats | qB feats]
        KT_sb = big.tile([P, B, T], MMDT)
        VT_sb = big.tile([P, B, T], MMDT)
        VA_sb = big.tile([P, B, NKB, 65], MMDT)   # [vA | ones]
        VB_sb = big.tile([P, B, NKB, P], MMDT)    # [0.. | ones@32 | 0.. | vB]

        nc.gpsimd.memset(VB_sb[:], 0.0)
        nc.vector.memset(VA_sb[:, :, :, 64:65], 1.0)
        nc.vector.memset(VB_sb[:, :, :, 32:33], 1.0)

        yts = {}
        pys = {}
        for b in range(B):
            # ------------- QKV projection (transposed outputs) -------------
            for mt in range(NMT_B):
                g = b * NMT_B + mt
                xm = xpool.tile([P, KO, MT], MMDT, tag="xm")
                if g == 0:
                    # per-chunk loads, w on the scalar queue in parallel with
                    # x on the sync queue: the k-th QKV matmul starts as soon
                    # as its own pair of slices lands.
                    for k in range(KO):
                        nc.scalar.dma_start(out=w_sb[:, k, :], in_=w3[:, k, :])
                        nc.sync.dma_start(out=xm[:, k, :],
                                          in_=xT3[:, k, 0:MT])
                else:
                    nc.sync.dma_start(out=xm[:, :, :],
                                      in_=xT3[:, :, g * MT:(g + 1) * MT])
                for nch in range(3):
                    pq = psum.tile([P, MT], F32, tag="py", bufs=4)
                    for k in range(KO):
                        nc.tensor.matmul(
                            pq[:],
                            (w_sb[:, k, nch * P:(nch + 1) * P]),
                            (xm[:, k, :]),
                            start=(k == 0),
                            stop=(k == KO - 1),
                        )
                    dst = (QT_sb, KT_sb, VT_sb)[nch]
                    dslice = dst[:, b, mt * MT:(mt + 1) * MT]
                    if nch < 2:
                        nc.vector.tensor_scalar_add(
                            out=dslice, in0=pq[:], scalar1=bqk_sb[:, nch:nch + 1]
                        )
                    else:
                        nc.vector.tensor_copy(out=dslice, in_=pq[:])

            # ------------- V back-transpose to [token, feat] -------------
            for kb in range(NKB):
                pt = psum.tile([P, P], MMDT, tag="py", bufs=4)
                nc.tensor.transpose(pt[:], VT_sb[:, b, kb * P:(kb + 1) * P], idb_sb[:])
                nc.vector.tensor_copy(out=VA_sb[:, b, kb, 0:64], in_=pt[:, 0:64])
                nc.vector.tensor_copy(out=VB_sb[:, b, kb, 64:128], in_=pt[:, 64:128])

            # ------------- causal attention -------------
            # One flat stream of score-groups for the whole batch, diagonal
            # blocks first within each qt. attnV trails scores by SKEW groups
            # ACROSS qt boundaries, so the PE never drains while waiting for
            # the exp/mask of the last blocks of a qt -- the next qt's score
            # matmuls fill the bubble.
            SKEW = 4

            def emit_normalize(qt, b=b):
                pyA, pyB = pys[(b, qt)]
                yu = spool.tile([P, 2 * QTW], F32, tag="yu", bufs=3,
                                name=f"yu_{b}_{qt}")
                nc.vector.tensor_copy(out=yu[0:65, 0:QTW], in_=pyA[0:65, :])
                nc.vector.tensor_copy(out=yu[0:128, QTW:2 * QTW], in_=pyB[:, :])
                # SBUF rows can't partition-broadcast directly; bounce the two
                # RAW denominator rows through DRAM, broadcast each across its
                # head's 64 partitions, then reciprocal the broadcast tile with
                # the fast custom-DVE approx (~0.8us on [128,512] vs 3.4us for
                # nc.vector.reciprocal on a single [1,512] row).
                dr = dpool.tile([2, QTW], F32, tag="dr", name=f"dr_{b}_{qt}")
                nc.sync.dma_start(out=dr[1:2, :], in_=yu[64:65, 0:QTW])
                nc.sync.dma_start(out=dr[0:1, :], in_=yu[32:33, QTW:2 * QTW])
                db = spool.tile([P, QTW], F32, tag="db", name=f"db_{b}_{qt}")
                rowB, rowA = dr[0:1, :], dr[1:2, :]
                srcA = bass.AP(tensor=rowA.tensor, offset=rowA.offset,
                               ap=[[0, 64], [1, QTW]])
                srcB = bass.AP(tensor=rowB.tensor, offset=rowB.offset,
                               ap=[[0, 64], [1, QTW]])
                nc.sync.dma_start(out=db[0:64, :], in_=srcA)
                nc.sync.dma_start(out=db[64:128, :], in_=srcB)
                rb = spool.tile([P, QTW], F32, tag="rb", name=f"rb_{b}_{qt}")
                nc.vector.reciprocal_approx_fast(out=rb[:, :], in_=db[:, :])
                yTq = ypool.tile([P, QTW], MMDT, tag="yT", name=f"yT_{b}_{qt}")
                yts[(b, qt)] = yTq
                nc.vector.tensor_tensor(
                    yTq[0:64, :], yu[0:64, 0:QTW], rb[0:64, :], mult)
                nc.vector.tensor_tensor(
                    yTq[64:128, :], yu[64:128, QTW:2 * QTW], rb[64:128, :], mult)

            def emit_attnv(item, b=b):
                qt, kb, e, qoff, first, last = item
                if first:
                    pyA = psum.tile([P, QTW], F32, tag="py", bufs=4,
                                    name=f"pyA_{b}_{qt}")
                    pyB = psum.tile([P, QTW], F32, tag="py", bufs=4,
                                    name=f"pyB_{b}_{qt}")
                    pys[(b, qt)] = (pyA, pyB)
                pyA, pyB = pys[(b, qt)]
                nc.tensor.matmul(
                    pyA[0:65, qoff:QTW], (VA_sb[:, b, kb, :]),
                    (e[:, qoff:QTW]),
                    start=first, stop=last, skip_group_check=True,
                )
                nc.tensor.matmul(
                    pyB[:, qoff:QTW], (VB_sb[:, b, kb, :]),
                    (e[:, QTW + qoff:2 * QTW]),
                    start=first, stop=last, skip_group_check=True,
                )
                if last:
                    emit_normalize(qt)

            groups = []
            for qt in range(NQT):
                order = list(range(qt * 4, (qt + 1) * 4)) + list(range(0, qt * 4))
                for i, kb in enumerate(order):
                    groups.append((qt, kb, i == 0, i == len(order) - 1))

            pend = []
            for (qt, kb, first, last) in groups:
                # diagonal blocks only touch q positions >= 128*d; narrow
                # all work (scores, exp, mask, attnV) to that window.
                d = kb - (qt * (QTW // P))
                qoff = 0 if os.environ.get("KERNEL_NARROW", "1") == "0" else max(0, d) * P
                w = QTW - qoff
                q0 = qt * QTW + qoff
                ps = psum.tile([P, 2 * QTW], F32, tag="ps", bufs=2,
                               name=f"ps_{b}_{qt}_{kb}")
                nc.tensor.matmul(
                    ps[:, qoff:QTW],
                    (KT_sb[0:64, b, kb * P:(kb + 1) * P]),
                    (QT_sb[0:64, b, q0:q0 + w]),
                    start=True, stop=True, tile_position=(0, 0),
                )
                nc.tensor.matmul(
                    ps[:, QTW + qoff:2 * QTW],
                    (KT_sb[64:128, b, kb * P:(kb + 1) * P]),
                    (QT_sb[64:128, b, q0:q0 + w]),
                    start=True, stop=True, tile_position=(64, 0),
                )
                e = epool.tile([P, 2 * QTW], MMDT, tag="e", bufs=SKEW + 2,
                               name=f"e_{b}_{qt}_{kb}")
                ps3 = ps.rearrange("p (h q) -> p h q", h=2)
                e3 = e.rearrange("p (h q) -> p h q", h=2)
                nc.scalar.activation(out=e3[:, :, qoff:], in_=ps3[:, :, qoff:],
                                     func=Exp, scale=SCALE)
                if d >= 0:
                    # within the window: keep exp[j, h, i'] where i' >= j
                    nc.gpsimd.affine_select(
                        out=e3[:, :, qoff:],
                        in_=e3[:, :, qoff:],
                        pattern=[[0, 2], [1, w]],
                        compare_op=mybir.AluOpType.is_ge,
                        fill=0.0,
                        base=0,
                        channel_multiplier=-1,
                    )
                pend.append((qt, kb, e, qoff, first, last))
                if len(pend) > SKEW:
                    emit_attnv(pend.pop(0))
            for item in pend:
                emit_attnv(item)

            # ------------- output projection (row-sharded partial) -------------
            # proj bias is added on the host after the partial sum; psum
            # evacuation alternates ACT/DVE per half so neither engine
            # serializes the proj phase. Partials are written bf16 (halves
            # the 16MB/core output DMA; host sums in f64).
            for sm in range(T // P):
                osb = opool.tile([P, C], BF16, tag="osb")
                for nh in range(C // QTW):
                    po = psum.tile([P, QTW], F32, tag="py", bufs=4)
                    yTq = yts[(b, sm // 4)]
                    nc.tensor.matmul(
                        po[:],
                        (yTq[:, (sm % 4) * P:(sm % 4 + 1) * P]),
                        (wp_sb[:, nh * QTW:(nh + 1) * QTW]),
                        start=True, stop=True,
                    )
                    if (sm + nh) % 2 == 0:
                        nc.scalar.copy(
                            out=osb[:, nh * QTW:(nh + 1) * QTW], in_=po[:])
                    else:
                        nc.vector.tensor_copy(
                            out=osb[:, nh * QTW:(nh + 1) * QTW], in_=po[:])
                r0 = b * T + sm * P
                # b0 outs ride the Pool SWDGE queue (sync is busy prefetching
                # b1's x tiles then); b1 outs ride the now-idle sync HW queue,
                # whose dispatch is ~2x faster -- this drains the kernel tail.
                if b == 0:
                    nc.gpsimd.dma_start(out=out2[r0:r0 + P, :], in_=osb[:])
                else:
                    nc.sync.dma_start(out=out2[r0:r0 + P, :], in_=osb[:])

    nc.finalize()
    return nc


def prep_inputs(x, W_qkv, b_qkv, W_proj, b_proj):
    """Host-side sharding: returns list of 8 per-core input dicts."""
    import ml_dtypes
    mmnp = np.float32 if _MM == "f32" else ml_dtypes.bfloat16
    x = np.asarray(x, dtype=np.float32)
    W_qkv = np.asarray(W_qkv, dtype=np.float32)
    b_qkv = np.asarray(b_qkv, dtype=np.float32)
    W_proj = np.asarray(W_proj, dtype=np.float32)
    b_proj = np.asarray(b_proj, dtype=np.float32)

    xT = np.ascontiguousarray(x.reshape(BT, C).T).astype(mmnp)   # [C, BT]
    ident = np.eye(P, dtype=np.float32)

    in_maps = []
    for c in range(NCORES):
        hA, hB = HPC * c, HPC * c + 1
        cols = []
        for part in range(3):                               # q, k, v
            for h in (hA, hB):
                cols.append(W_qkv[:, part * C + h * D: part * C + (h + 1) * D])
        w = np.ascontiguousarray(np.concatenate(cols, axis=1)).astype(mmnp)  # [C, 384]

        bq = np.concatenate([b_qkv[hA * D:(hA + 1) * D], b_qkv[hB * D:(hB + 1) * D]])
        bk = np.concatenate([b_qkv[C + hA * D: C + (hA + 1) * D],
                             b_qkv[C + hB * D: C + (hB + 1) * D]])
        bv = np.concatenate([b_qkv[2 * C + hA * D: 2 * C + (hA + 1) * D],
                             b_qkv[2 * C + hB * D: 2 * C + (hB + 1) * D]])
        bqk = np.ascontiguousarray(np.stack([bq, bk], axis=1))  # [128, 2]

        wp = np.ascontiguousarray(W_proj[c * P:(c + 1) * P, :]).astype(mmnp)

        in_maps.append({
            "xT": xT,
            "w": w,
            "bqk": bqk,
            "wp": wp,
            "ident": ident,
        })
    return in_maps


_NC_CACHE = None


def kernel(x, W_qkv, b_qkv, W_proj, b_proj):
    global _NC_CACHE, LAST_RESULT
    from concourse.bass_utils import run_bass_kernel_spmd

    if _NC_CACHE is None:
        _NC_CACHE = build_nc()
    nc = _NC_CACHE

    in_maps = prep_inputs(x, W_qkv, b_qkv, W_proj, b_proj)
    trace = os.environ.get("KERNEL_TRACE", "0") == "1"
    res = run_bass_kernel_spmd(nc, in_maps, list(range(NCORES)), trace=trace)
    LAST_RESULT = res

    acc = np.zeros((BT, C), dtype=np.float64)
    for r in res.results:
        acc += r["out"].astype(np.float64)
    # attn rows sum to 1, so the V bias contributes b_v @ W_proj to every
    # token row; add it and the proj bias here (exact, part of unshard).
    W_proj = np.asarray(W_proj, dtype=np.float32)
    b_qkv = np.asarray(b_qkv, dtype=np.float32)
    b_proj = np.asarray(b_proj, dtype=np.float32)
    acc += (b_qkv[2 * C:].astype(np.float64) @ W_proj.astype(np.float64)
            + b_proj.astype(np.float64))
    return acc.astype(np.float32).reshape(B, T, C)

